# revision 26
# baseline (speedup 1.0000x reference)
"""AutoAdaptiveFocalLossV2 on 8 Trainium2 NeuronCores.

Math per row r of input [N, C]:
    s      = sum_c exp(x[r, c])
    logpt  = x[r, target[r]] - log(s)
    pt     = exp(logpt)
    gamma  = g[searchsorted(edges, pt)]
    loss_r = -(1 - pt + 1e-20)^gamma * logpt
Output = sum_r loss_r.  Sharding: pure data-parallel, 16384 rows/core.

The rel-err budget (2e-2) allows low-precision streaming; the row-sum
is the only per-element work and is spread over THREE reducers:
  - A-blocks (51/128, ACT): streamed as fp8_e4m3 (1 B/elem; ACT rate is
    dtype-independent); activation(Exp, accum_out) does exp+row-sum
    (~1.41 us/block measured incl. the accumulator read).
  - D-blocks (29/128, DVE): streamed as bf16; a Schraudolph exp2 --
    tensor_scalar computes int16(x*(2^7*log2e) + (127*2^7 + adj)) whose
    bit pattern IS bf16(exp(x)) to +-3% (sawtooth centered by adj;
    total-loss error ~1e-5); int16 output engages the DVE 4x perf mode.
    One grouped tensor_reduce per chunk row-sums the chunk (1x rate).
  - T-lane (48/128, PE): the same rows TRANSPOSED on the host (c on
    partitions, padded to 1024 with -300 -> exp underflows to -0.0).
    12 groups of 512 rows stream as [128, 8, 512] bf16 tiles; one DVE
    4x Schraudolph per group, then 8 PE matmuls whose stationary
    weights are ONE-HOT COLUMNS (lhsT[:, g] = ones) so group g's
    column-sums land on PSUM PARTITION g -- 96 matmuls accumulate the
    whole lane's row-sums into a single [12, 512] PSUM bank. One DVE
    copy evacuates it, and a DRAM round-trip re-tiles the 24 KB of sums
    to the [128, 48] staging layout (a flat DRAM view is the only legal
    way to re-partition on this DMA).
The target logit x[r, t[r]] is gathered on the host (index-driven data
movement, like the resharding) and DMA'd as a tiny fp32 side tensor.

ACT and DVE write row-sums into separate staging tiles; the host
permutes row-blocks so staging position and the xt layout agree. Both
streams are packed per chunk so each partition's chunk data is one
contiguous descriptor. The binning epilogue runs in halves: half 0 =
stream columns 0..63, its serial chain spread piecewise over chunks
16..19 so each cross-engine hop overlaps stream work; half 1 = the
remaining 16 stream columns + the 48 T-lane columns, as kernel tail.
The 14-bin gamma mask loop stays on DVE (Pool's ~1.1us/op fixed cost
makes its serial chains a latency bomb -- measured).

The single-sync-wait workarounds (sink ops, post-pass wait rewrites)
exist because this walrus build refuses any instruction carrying more
than one semaphore wait; verify_single_wait() enforces it at build.
"""

import os
import numpy as np

N = 131072
C = 1000
NUM_BINS = 15
P = 128
NCORES = 8
RPC = N // NCORES          # 16384 rows per core
COLS = 128                 # row-blocks per core
SCOLS = 80                 # row-blocks in the chunked streams
J = 4                      # row-blocks per chunk
CHUNKS = SCOLS // J        # 20 chunks per core
# T-lane: last 48 row-blocks, transposed, PE-reduced
TBLK = COLS - SCOLS        # 48 blocks = 6144 rows
TROWS = TBLK * P
TG = TROWS // 512          # 12 groups of 512 rows
CG = 8                     # c padded to 1024 = 8 groups of 128
CPAD = CG * P

N_A, N_D = 51, 29          # stream split: ACT fp8 / DVE bf16

# Schraudolph exp2-in-bf16-bits constants.
SCH_C1 = 1.4426950408889634 * 128.0            # log2(e) * 2^7
SCH_C2 = 127.0 * 128.0 - 7.4                   # bias + sawtooth centering

LAST_RESULT = None


def build_assignment():
    """Stream-column engine assignment (largest-remainder interleave),
    reordered within each chunk to [D..., A...]."""
    quota = {"A": N_A, "D": N_D}
    placed = {"A": 0, "D": 0}
    assign = []
    for c in range(SCOLS):
        e = max("AD", key=lambda e: (quota[e] * (c + 1) - placed[e] * SCOLS,
                                     quota[e]))
        assign.append(e)
        placed[e] += 1
    out = []
    for k in range(CHUNKS):
        ch = assign[k * J : (k + 1) * J]
        out.extend(["D"] * ch.count("D") + ["A"] * ch.count("A"))
    assert out.count("A") == N_A and out.count("D") == N_D
    return out


ASSIGN = build_assignment()
REGION = ["A" if e == "A" else "V" for e in ASSIGN]
ENG_IDX = []
_c = {"A": 0, "V": 0}
for _r in REGION:
    ENG_IDX.append(_c[_r])
    _c[_r] += 1
# epilogue halves: half 0 = stream cols [0:64); half 1 = stream cols
# [64:80) followed by the 48 T-lane columns.
SEG_LO = [0, 64, SCOLS]
QTR_CT = []
for q in range(2):
    seg = REGION[SEG_LO[q] : SEG_LO[q + 1]]
    QTR_CT.append({r: seg.count(r) for r in "AV"})
QTR_BASE = [{r: sum(QTR_CT[i][r] for i in range(q)) for r in "AV"}
            for q in range(2)]
H1_STREAM = SEG_LO[2] - SEG_LO[1]      # 16 stream cols in half 1


def epilogue_col(c):
    """Epilogue position of stream column c: [A][V] regions per half."""
    r, m = REGION[c], ENG_IDX[c]
    q = 0 if c < 64 else 1
    m_in_q = m - QTR_BASE[q][r]
    base = 0 if r == "A" else QTR_CT[q]["A"]
    return q * 64 + base + m_in_q


def build_program(bin_edges, bin_gammas, hw_fixups=True):
    import concourse.bass as bass
    import concourse.mybir as mybir
    import concourse.tile as tile

    f32 = mybir.dt.float32
    bf16 = mybir.dt.bfloat16
    fp8 = mybir.dt.float8e4
    i16 = mybir.dt.int16
    Alu = mybir.AluOpType
    Act = mybir.ActivationFunctionType

    edges = [float(v) for v in np.asarray(bin_edges, np.float64)]
    gammas = [float(v) for v in np.asarray(bin_gammas, np.float64)]
    assert len(edges) == NUM_BINS - 1 and len(gammas) == NUM_BINS

    nc = bass.Bass()
    x8_d = nc.dram_tensor("x8", [N_A * P * C], fp8, kind="ExternalInput")
    x16_d = nc.dram_tensor("x16", [N_D * P * C], bf16, kind="ExternalInput")
    xT_d = nc.dram_tensor("xT", [TG * P * CG * 512], bf16,
                          kind="ExternalInput")
    xt_d = nc.dram_tensor("xt", [P, COLS], f32, kind="ExternalInput")
    scr_d = nc.dram_tensor("scr", [TROWS], f32, kind="Internal")
    out_d = nc.dram_tensor("out", [P, 1], f32, kind="ExternalOutput")

    a_start = [0] * (CHUNKS + 1)
    b_start = [0] * (CHUNKS + 1)
    for k in range(CHUNKS):
        ch = ASSIGN[k * J : (k + 1) * J]
        a_start[k + 1] = a_start[k] + ch.count("A")
        b_start[k + 1] = b_start[k] + J - ch.count("A")

    with tile.TileContext(nc) as tc:
        with (
            tc.tile_pool(name="x8pool", bufs=10) as x8pool,
            tc.tile_pool(name="x16pool", bufs=10) as x16pool,
            tc.tile_pool(name="xTpool", bufs=4) as xTpool,
            tc.tile_pool(name="iTpool", bufs=2) as iTpool,
            tc.tile_pool(name="scratch", bufs=2) as scratch,
            tc.tile_pool(name="epool", bufs=2) as epool,
            tc.tile_pool(name="ipoolD", bufs=2) as ipoolD,
            tc.tile_pool(name="consts", bufs=1) as consts,
            tc.tile_pool(name="stage", bufs=1) as stage,
            tc.psum_pool(name="ppool", bufs=1) as ppool,
        ):
            ones = consts.tile([P, 1], f32, tag="ones")
            nc.vector.memset(ones[:], 1.0)
            # one-hot stationary weights: wts[:, g, m] = 1 iff m == g
            wts = consts.tile([P, TG, TG], bf16, tag="wts")
            nc.vector.memset(wts[:], 0.0)
            for g in range(TG):
                nc.vector.memset(wts[:, g, g : g + 1], 1.0)

            s_acc = stage.tile([P, N_A], f32, tag="s_acc")
            s_vec = stage.tile([P, N_D], f32, tag="s_vec")
            xt_all = stage.tile([P, COLS], f32, tag="xt_all")
            psum_t = ppool.tile([TG, 512], f32, tag="psum")

            nc.sync.dma_start(out=xt_all[:], in_=xt_d[:, :])
            sink0 = scratch.tile([P, 1], f32, tag="sink")
            nc.vector.tensor_tensor(
                out=sink0[:], in0=xt_all[:, 0:1], in1=ones[:], op=Alu.add
            )

            def dve_absorb(src_ap):
                t = scratch.tile([P, 1], f32, tag="eabs")
                nc.vector.tensor_tensor(
                    out=t[:], in0=src_ap, in1=ones[:], op=Alu.add
                )

            parts = stage.tile([P, 2], f32, tag="parts")
            s_all_t = [stage.tile([P, 64], f32, name=f"s_all{q}",
                                  tag=f"s_all{q}") for q in range(2)]
            ep = {}

            def ep_merge_ln(q):
                ca, cv = QTR_CT[q]["A"], QTR_CT[q]["V"]
                sa_lo, sv_lo = QTR_BASE[q]["A"], QTR_BASE[q]["V"]
                s_all = s_all_t[q]
                nc.scalar.copy(s_all[:, 0:ca], s_acc[:, sa_lo : sa_lo + ca])
                nc.scalar.copy(
                    s_all[:, ca : ca + cv], s_vec[:, sv_lo : sv_lo + cv]
                )
                # (for q == 1 the T-lane sums were already DMA'd into
                # s_all[:, 16:64])
                ln_s = stage.tile([P, 64], f32, tag=f"ln_s{q}")
                nc.scalar.activation(out=ln_s[:], in_=s_all[:], func=Act.Ln)
                ep[q] = dict(ln_s=ln_s)

            def ep_logpt(q):
                ln_s = ep[q]["ln_s"]
                logpt = stage.tile([P, 64], f32, tag=f"logpt{q}")
                dve_absorb(ln_s[:, 0:1])
                nc.vector.tensor_tensor(
                    out=logpt[:], in0=xt_all[:, q * 64 : (q + 1) * 64],
                    in1=ln_s[:], op=Alu.subtract,
                )
                ep[q]["logpt"] = logpt

            def ep_pt(q):
                logpt = ep[q]["logpt"]
                pt = stage.tile([P, 64], f32, tag=f"pt{q}")
                nc.scalar.activation(out=pt[:], in_=logpt[:], func=Act.Exp)
                om = stage.tile([P, 64], f32, tag=f"om{q}")  # 1 - pt
                nc.scalar.activation(
                    out=om[:], in_=pt[:], func=Act.Copy, scale=-1.0, bias=1.0
                )
                ln1m = stage.tile([P, 64], f32, tag=f"ln1m{q}")
                nc.scalar.activation(out=ln1m[:], in_=om[:], func=Act.Ln)
                gamma = stage.tile([P, 64], f32, tag=f"gamma{q}")
                mtmp = stage.tile([P, 64], f32, tag=f"mtmp{q}")
                ep[q].update(pt=pt, ln1m=ln1m, gamma=gamma, mtmp=mtmp)

            def ep_gamma(q, bins):
                st = ep[q]
                if bins[0] == 0:
                    nc.vector.memset(st["gamma"][:], gammas[0])
                for i in bins:
                    dg = gammas[i + 1] - gammas[i]
                    nc.vector.tensor_scalar(
                        out=st["mtmp"][:], in0=st["pt"][:],
                        scalar1=edges[i], scalar2=dg,
                        op0=Alu.is_ge, op1=Alu.mult,
                    )
                    nc.vector.tensor_tensor(
                        out=st["gamma"][:], in0=st["gamma"][:],
                        in1=st["mtmp"][:], op=Alu.add,
                    )

            def ep_suffix(q):
                st = ep[q]
                prod = stage.tile([P, 64], f32, tag=f"prod{q}")
                dve_absorb(st["ln1m"][:, 0:1])
                nc.vector.tensor_tensor(
                    out=prod[:], in0=st["gamma"][:], in1=st["ln1m"][:],
                    op=Alu.mult,
                )
                focal = stage.tile([P, 64], f32, tag=f"focal{q}")
                nc.scalar.activation(out=focal[:], in_=prod[:], func=Act.Exp)
                contrib = stage.tile([P, 64], f32, tag=f"contrib{q}")
                dve_absorb(focal[:, 0:1])
                nc.vector.tensor_tensor(
                    out=contrib[:], in0=focal[:], in1=st["logpt"][:],
                    op=Alu.mult,
                )
                nc.vector.tensor_reduce(
                    out=parts[:, q : q + 1], in_=contrib[:],
                    axis=mybir.AxisListType.X, op=Alu.add, negate=True,
                )

            def t_group(g):
                xT_t = xTpool.tile([P, CG, 512], bf16, tag="xT")
                src = xT_d[g * P * CG * 512 : (g + 1) * P * CG * 512]
                nc.sync.dma_start(
                    out=xT_t[:], in_=src.rearrange("(p x) -> p x", p=P)
                )
                # DVE sink absorbs the DMA wait so the tensor_scalar can
                # carry just its iT-recycle (PE) wait.
                sk = scratch.tile([P, 1], f32, tag="sink")
                nc.vector.tensor_tensor(
                    out=sk[:], in0=xT_t[:, 0, 0:2].bitcast(f32),
                    in1=ones[:], op=Alu.add,
                )
                iT_t = iTpool.tile([P, CG, 512], i16, tag="iT")
                nc.vector.tensor_scalar(
                    out=iT_t[:], in0=xT_t[:],
                    scalar1=SCH_C1, scalar2=SCH_C2,
                    op0=Alu.mult, op1=Alu.add,
                )
                for cg in range(CG):
                    nc.tensor.matmul(
                        psum_t[:],
                        wts[:, g, :],
                        iT_t[:, cg, :].bitcast(bf16),
                        start=(g == 0 and cg == 0),
                        stop=(g == TG - 1 and cg == CG - 1),
                    )

            for k in range(CHUNKS):
                ch = ASSIGN[k * J : (k + 1) * J]
                nd_k = ch.count("D")
                na_k = ch.count("A")
                if na_k:
                    x8_t = x8pool.tile([P, 3, C], fp8, tag="x8")
                    src = x8_d[a_start[k] * P * C : a_start[k + 1] * P * C]
                    src = src.rearrange("(p x) -> p x", p=P)
                    nc.sync.dma_start(out=x8_t[:, 0:na_k, :], in_=src)
                if nd_k:
                    x16_t = x16pool.tile([P, 2, C], bf16, tag="x16")
                    src = x16_d[b_start[k] * P * C : b_start[k + 1] * P * C]
                    src = src.rearrange("(p x) -> p x", p=P)
                    nc.sync.dma_start(out=x16_t[:, 0:nd_k, :], in_=src)
                    e_t = ipoolD.tile([P, 2, C], i16, tag="schD")
                    for j in range(nd_k):
                        nc.vector.tensor_scalar(
                            out=e_t[:, j, :], in0=x16_t[:, j, :],
                            scalar1=SCH_C1, scalar2=SCH_C2,
                            op0=Alu.mult, op1=Alu.add,
                        )
                    m0 = ENG_IDX[k * J]
                    nc.vector.tensor_reduce(
                        out=s_vec[:, m0 : m0 + nd_k],
                        in_=e_t[:, 0:nd_k, :].bitcast(bf16),
                        axis=mybir.AxisListType.X, op=Alu.add,
                    )
                for j in range(na_k):
                    m = ENG_IDX[k * J + nd_k + j]
                    dump = epool.tile([P, C], bf16, tag="exp_out")
                    nc.scalar.activation(
                        out=dump[:], in_=x8_t[:, j, :], func=Act.Exp,
                        accum_out=s_acc[:, m : m + 1],
                    )
                if 2 <= k < TG + 2:
                    t_group(k - 2)
                if k == 16:
                    ep_merge_ln(0)
                elif k == 17:
                    ep_logpt(0)
                    ep_pt(0)
                elif k in (18, 19):
                    ep_gamma(0, range(7 * (k - 18), 7 * (k - 17)))

            ep_suffix(0)
            # T-lane: evacuate PSUM, re-tile via DRAM into half-1 staging
            flat_T = stage.tile([TG, 512], f32, tag="flatT")
            nc.vector.tensor_copy(out=flat_T[:], in_=psum_t[:])
            nc.sync.dma_start(
                out=scr_d[:].rearrange("(g f) -> g f", g=TG), in_=flat_T[:]
            )
            nc.sync.dma_start(
                out=s_all_t[1][:, H1_STREAM:64],
                in_=scr_d[:].rearrange("(p m) -> p m", p=P),
            )
            ep_merge_ln(1)
            ep_logpt(1)
            ep_pt(1)
            ep_gamma(1, range(NUM_BINS - 1))
            ep_suffix(1)
            part = stage.tile([P, 1], f32, tag="part")
            nc.vector.tensor_tensor(
                out=part[:], in0=parts[:, 0:1], in1=parts[:, 1:2], op=Alu.add
            )
            nc.sync.dma_start(out=out_d[:, :], in_=part[:])

    if hw_fixups:
        apply_hw_fixups(nc, mybir)
        verify_single_wait(nc)
    return nc


def apply_hw_fixups(nc, mybir):
    # Strip redundant own-engine waits (in-order queues make them no-ops).
    own_prefix = {
        "EngineType.DVE": "DVE",
        "EngineType.Activation": "Activation",
        "EngineType.Pool": "Pool",
        "EngineType.PE": "PE",
        "EngineType.SP": "SP",
    }
    for blk in nc.m.functions[0].blocks:
        for ins in blk.instructions:
            si = getattr(ins, "sync_info", None)
            if si is None or type(ins).__name__ == "InstDMACopy":
                continue
            if len(si.on_wait) <= 1:
                continue
            pref = own_prefix.get(str(getattr(ins, "engine", "")), None)
            if pref is None:
                continue
            keep = [w for w in si.on_wait if not w.ant_name.startswith(pref + "_")]
            if len(keep) < len(si.on_wait):
                ins.sync_info = type(si)(on_wait=keep, on_update=list(si.on_update))

    # Structural two-wait cases with a transitive single-wait replacement:
    #  - DVE tensor_scalar {PE WAR, DMA RAW} (T-lane): the sink absorbed
    #    the DMA wait for the DVE queue; keep the PE wait.
    for blk in nc.m.functions[0].blocks:
        for ins in blk.instructions:
            si = getattr(ins, "sync_info", None)
            if si is None or type(ins).__name__ == "InstDMACopy":
                continue
            if len(si.on_wait) <= 1:
                continue
            eng = str(getattr(ins, "engine", ""))
            names = [w.ant_name for w in si.on_wait]
            if (
                eng == "EngineType.DVE"
                and len(si.on_wait) == 2
                and any(n.startswith("PE") for n in names)
                and any(n.startswith("DMA") for n in names)
            ):
                keep = [w for w in si.on_wait if w.ant_name.startswith("PE")]
                ins.sync_info = type(si)(on_wait=keep, on_update=list(si.on_update))

    # walrus' DMA encoding holds a single sync wait. The last reader of a
    # stream slot is a DVE op for x16/xT and an ACT op for x8 (the T-lane
    # xT slots are last read by the PE matmuls, whose wait implies the DVE
    # tensor_scalar and its DMA wait).
    for blk in nc.m.functions[0].blocks:
        for ins in blk.instructions:
            si = getattr(ins, "sync_info", None)
            if si is None or type(ins).__name__ != "InstDMACopy":
                continue
            if len(si.on_wait) <= 1:
                continue
            own_lane = si.on_update[0].ant_name if si.on_update else ""
            keep = (
                [w for w in si.on_wait if w.ant_name.startswith("PE")]
                or [w for w in si.on_wait if w.ant_name.startswith("DVE")]
                or [w for w in si.on_wait if w.ant_name.startswith("Activation")]
                # DMA-producer RAW (e.g. the DRAM re-tile bounce): keep the
                # foreign lane, drop the own-lane ordering wait (increments
                # are atomic adds; issue order per queue is FIFO anyway).
                or [w for w in si.on_wait if w.ant_name != own_lane]
            )
            assert len(keep) == 1, (ins.name, [w.ant_name for w in si.on_wait])
            ins.sync_info = type(si)(on_wait=keep, on_update=list(si.on_update))

    # Split multi-wait kernel-tail drains into single-wait chains.
    for blk in nc.m.functions[0].blocks:
        il = blk.instructions
        i = 0
        while i < len(il):
            ins = il[i]
            si = getattr(ins, "sync_info", None)
            if (
                si is not None
                and type(ins).__name__ == "InstDrain"
                and len(si.on_wait) > 1
            ):
                SyncInfo = type(si)
                waits = list(si.on_wait)
                for k, w in enumerate(waits[:-1]):
                    d = mybir.InstDrain(
                        name=f"{ins.name}-w{k}", ins=[], outs=[],
                        bass_is_fusable=False,
                    )
                    d.engine = ins.engine
                    d.sync_info = SyncInfo(on_wait=[w], on_update=[])
                    il.insert(i, d)
                    i += 1
                ins.sync_info = SyncInfo(
                    on_wait=[waits[-1]], on_update=list(si.on_update)
                )
            i += 1


def verify_single_wait(nc):
    """Build-time check of the walrus single-wait constraint."""
    bad = []
    for blk in nc.m.functions[0].blocks:
        for ins in blk.instructions:
            si = getattr(ins, "sync_info", None)
            if si is not None and len(si.on_wait) > 1:
                bad.append(
                    (ins.name, type(ins).__name__,
                     str(getattr(ins, "engine", "")),
                     [w.ant_name for w in si.on_wait])
                )
    assert not bad, f"multi-wait instructions after fixups: {bad}"


def make_in_maps(input, target):
    import ml_dtypes

    x = np.asarray(input, dtype=np.float32)
    t = np.asarray(target).astype(np.int64)
    xt = x[np.arange(N), t]

    in_maps = []
    for core in range(NCORES):
        xs = x[core * RPC : (core + 1) * RPC]
        blocks = xs.reshape(COLS, P, C)
        # chunked streams over the first SCOLS row-blocks
        x8_parts, x16_parts = [], []
        for k in range(CHUNKS):
            ch = ASSIGN[k * J : (k + 1) * J]
            cols = list(range(k * J, (k + 1) * J))
            a_cols = [c for c, e in zip(cols, ch) if e == "A"]
            b_cols = [c for c, e in zip(cols, ch) if e != "A"]
            if a_cols:
                x8_parts.append(blocks[a_cols].transpose(1, 0, 2).reshape(-1))
            if b_cols:
                x16_parts.append(blocks[b_cols].transpose(1, 0, 2).reshape(-1))
        x8 = np.concatenate(x8_parts).astype(ml_dtypes.float8_e4m3)
        x16 = np.concatenate(x16_parts).astype(ml_dtypes.bfloat16)
        # T-lane: last TBLK row-blocks transposed, c padded to 1024
        xpad = np.full((TROWS, CPAD), -300.0, np.float32)
        xpad[:, :C] = xs[SCOLS * P :]
        xT = np.ascontiguousarray(
            xpad.T.reshape(CG, P, TG, 512).transpose(2, 1, 0, 3)
        ).astype(ml_dtypes.bfloat16)  # [g, p, cg, f]
        # xt layout matching the epilogue staging
        xts = np.zeros((P, COLS), np.float32)
        xt_core = xt[core * RPC : (core + 1) * RPC]
        xt_blk = xt_core[: SCOLS * P].reshape(SCOLS, P)
        for c in range(SCOLS):
            xts[:, epilogue_col(c)] = xt_blk[c]
        # T-row i sits at s_all1[i // TBLK, H1_STREAM + i % TBLK]
        xt_T = xt_core[SCOLS * P :].reshape(P, TBLK)
        xts[:, 64 + H1_STREAM : 128] = xt_T
        in_maps.append({"x8": x8, "x16": x16, "xT": xT.reshape(-1),
                        "xt": xts})
    return in_maps


def kernel(input, target, bin_edges, bin_gammas):
    global LAST_RESULT
    from concourse.bass_utils import run_bass_kernel_spmd

    nc = build_program(bin_edges, bin_gammas)
    in_maps = make_in_maps(input, target)
    trace = bool(os.environ.get("BASS_TRACE"))
    res = run_bass_kernel_spmd(nc, in_maps, list(range(NCORES)), trace=trace)
    LAST_RESULT = res
    total = np.float64(0.0)
    for r in res.results:
        total += r["out"].astype(np.float64).sum()
    return np.float32(total)


# revision 27
# speedup vs baseline: 1.0100x; 1.0100x over previous
"""AutoAdaptiveFocalLossV2 on 8 Trainium2 NeuronCores.

Math per row r of input [N, C]:
    s      = sum_c exp(x[r, c])
    logpt  = x[r, target[r]] - log(s)
    pt     = exp(logpt)
    gamma  = g[searchsorted(edges, pt)]
    loss_r = -(1 - pt + 1e-20)^gamma * logpt
Output = sum_r loss_r.  Sharding: pure data-parallel, 16384 rows/core.

The rel-err budget (2e-2) allows low-precision streaming; the row-sum
is the only per-element work and is spread over THREE reducers:
  - A-blocks (51/128, ACT): streamed as fp8_e4m3 (1 B/elem; ACT rate is
    dtype-independent); activation(Exp, accum_out) does exp+row-sum
    (~1.41 us/block measured incl. the accumulator read).
  - D-blocks (29/128, DVE): streamed as bf16; a Schraudolph exp2 --
    tensor_scalar computes int16(x*(2^7*log2e) + (127*2^7 + adj)) whose
    bit pattern IS bf16(exp(x)) to +-3% (sawtooth centered by adj;
    total-loss error ~1e-5); int16 output engages the DVE 4x perf mode.
    One grouped tensor_reduce per chunk row-sums the chunk (1x rate).
  - T-lane (48/128, PE): the same rows TRANSPOSED on the host (c on
    partitions, padded to 1024 with -300 -> exp underflows to -0.0).
    12 groups of 512 rows stream as [128, 8, 512] bf16 tiles; one DVE
    4x Schraudolph per group, then 8 PE matmuls whose stationary
    weights are ONE-HOT COLUMNS (lhsT[:, g] = ones) so group g's
    column-sums land on PSUM PARTITION g -- 96 matmuls accumulate the
    whole lane's row-sums into a single [12, 512] PSUM bank. One DVE
    copy evacuates it, and a DRAM round-trip re-tiles the 24 KB of sums
    to the [128, 48] staging layout (a flat DRAM view is the only legal
    way to re-partition on this DMA).
The target logit x[r, t[r]] is gathered on the host (index-driven data
movement, like the resharding) and DMA'd as a tiny fp32 side tensor.

ACT and DVE write row-sums into separate staging tiles; the host
permutes row-blocks so staging position and the xt layout agree. Both
streams are packed per chunk so each partition's chunk data is one
contiguous descriptor. The binning epilogue runs in halves: half 0 =
stream columns 0..63, its serial chain spread piecewise over chunks
16..19 so each cross-engine hop overlaps stream work; half 1 = the
remaining 16 stream columns + the 48 T-lane columns, as kernel tail.
The 14-bin gamma mask loop stays on DVE (Pool's ~1.1us/op fixed cost
makes its serial chains a latency bomb -- measured).

The single-sync-wait workarounds (sink ops, post-pass wait rewrites)
exist because this walrus build refuses any instruction carrying more
than one semaphore wait; verify_single_wait() enforces it at build.
"""

import os
import numpy as np

N = 131072
C = 1000
NUM_BINS = 15
P = 128
NCORES = 8
RPC = N // NCORES          # 16384 rows per core
COLS = 128                 # row-blocks per core
SCOLS = 80                 # row-blocks in the chunked streams
J = 4                      # row-blocks per chunk
CHUNKS = SCOLS // J        # 20 chunks per core
# T-lane: last 48 row-blocks, transposed, PE-reduced
TBLK = COLS - SCOLS        # 48 blocks = 6144 rows
TROWS = TBLK * P
TG = TROWS // 512          # 12 groups of 512 rows
CG = 8                     # c padded to 1024 = 8 groups of 128
CPAD = CG * P

N_A, N_D = 51, 29          # stream split: ACT fp8 / DVE bf16

# Schraudolph exp2-in-bf16-bits constants.
SCH_C1 = 1.4426950408889634 * 128.0            # log2(e) * 2^7
SCH_C2 = 127.0 * 128.0 - 7.4                   # bias + sawtooth centering

LAST_RESULT = None


def build_assignment():
    """Stream-column engine assignment (largest-remainder interleave),
    reordered within each chunk to [D..., A...]."""
    quota = {"A": N_A, "D": N_D}
    placed = {"A": 0, "D": 0}
    assign = []
    for c in range(SCOLS):
        e = max("AD", key=lambda e: (quota[e] * (c + 1) - placed[e] * SCOLS,
                                     quota[e]))
        assign.append(e)
        placed[e] += 1
    out = []
    for k in range(CHUNKS):
        ch = assign[k * J : (k + 1) * J]
        out.extend(["D"] * ch.count("D") + ["A"] * ch.count("A"))
    assert out.count("A") == N_A and out.count("D") == N_D
    return out


ASSIGN = build_assignment()
REGION = ["A" if e == "A" else "V" for e in ASSIGN]
ENG_IDX = []
_c = {"A": 0, "V": 0}
for _r in REGION:
    ENG_IDX.append(_c[_r])
    _c[_r] += 1
# epilogue halves: half 0 = stream cols [0:64); half 1 = stream cols
# [64:80) followed by the 48 T-lane columns.
SEG_LO = [0, 64, SCOLS]
QTR_CT = []
for q in range(2):
    seg = REGION[SEG_LO[q] : SEG_LO[q + 1]]
    QTR_CT.append({r: seg.count(r) for r in "AV"})
QTR_BASE = [{r: sum(QTR_CT[i][r] for i in range(q)) for r in "AV"}
            for q in range(2)]
H1_STREAM = SEG_LO[2] - SEG_LO[1]      # 16 stream cols in half 1


def epilogue_col(c):
    """Epilogue position of stream column c: [A][V] regions per half."""
    r, m = REGION[c], ENG_IDX[c]
    q = 0 if c < 64 else 1
    m_in_q = m - QTR_BASE[q][r]
    base = 0 if r == "A" else QTR_CT[q]["A"]
    return q * 64 + base + m_in_q


def build_program(bin_edges, bin_gammas, hw_fixups=True):
    import concourse.bass as bass
    import concourse.mybir as mybir
    import concourse.tile as tile

    f32 = mybir.dt.float32
    bf16 = mybir.dt.bfloat16
    fp8 = mybir.dt.float8e4
    i16 = mybir.dt.int16
    Alu = mybir.AluOpType
    Act = mybir.ActivationFunctionType

    edges = [float(v) for v in np.asarray(bin_edges, np.float64)]
    gammas = [float(v) for v in np.asarray(bin_gammas, np.float64)]
    assert len(edges) == NUM_BINS - 1 and len(gammas) == NUM_BINS

    nc = bass.Bass()
    x8_d = nc.dram_tensor("x8", [N_A * P * C], fp8, kind="ExternalInput")
    x16_d = nc.dram_tensor("x16", [N_D * P * C], bf16, kind="ExternalInput")
    xT_d = nc.dram_tensor("xT", [TG * P * CG * 512], bf16,
                          kind="ExternalInput")
    xt_d = nc.dram_tensor("xt", [P, COLS], f32, kind="ExternalInput")
    scr_d = nc.dram_tensor("scr", [TROWS], f32, kind="Internal")
    out_d = nc.dram_tensor("out", [P, 1], f32, kind="ExternalOutput")

    a_start = [0] * (CHUNKS + 1)
    b_start = [0] * (CHUNKS + 1)
    for k in range(CHUNKS):
        ch = ASSIGN[k * J : (k + 1) * J]
        a_start[k + 1] = a_start[k] + ch.count("A")
        b_start[k + 1] = b_start[k] + J - ch.count("A")

    with tile.TileContext(nc) as tc:
        with (
            tc.tile_pool(name="x8pool", bufs=10) as x8pool,
            tc.tile_pool(name="x16pool", bufs=10) as x16pool,
            tc.tile_pool(name="xTpool", bufs=3) as xTpool,
            tc.tile_pool(name="iTpool", bufs=2) as iTpool,
            tc.tile_pool(name="scratch", bufs=2) as scratch,
            tc.tile_pool(name="epool", bufs=2) as epool,
            tc.tile_pool(name="ipoolD", bufs=2) as ipoolD,
            tc.tile_pool(name="consts", bufs=1) as consts,
            tc.tile_pool(name="stage", bufs=1) as stage,
            tc.psum_pool(name="ppool", bufs=1) as ppool,
        ):
            ones = consts.tile([P, 1], f32, tag="ones")
            nc.vector.memset(ones[:], 1.0)
            # one-hot stationary weights: wts[:, g, m] = 1 iff m == g
            wts = consts.tile([P, TG, TG], bf16, tag="wts")
            nc.vector.memset(wts[:], 0.0)
            for g in range(TG):
                nc.vector.memset(wts[:, g, g : g + 1], 1.0)

            s_acc = stage.tile([P, N_A], f32, tag="s_acc")
            s_vec = stage.tile([P, N_D], f32, tag="s_vec")
            xt_all = stage.tile([P, COLS], f32, tag="xt_all")
            psum_t = ppool.tile([TG, 512], f32, tag="psum")

            nc.sync.dma_start(out=xt_all[:], in_=xt_d[:, :])
            sink0 = scratch.tile([P, 1], f32, tag="sink")
            nc.vector.tensor_tensor(
                out=sink0[:], in0=xt_all[:, 0:1], in1=ones[:], op=Alu.add
            )

            def dve_absorb(src_ap):
                t = scratch.tile([P, 1], f32, tag="eabs")
                nc.vector.tensor_tensor(
                    out=t[:], in0=src_ap, in1=ones[:], op=Alu.add
                )

            parts = stage.tile([P, 2], f32, tag="parts")
            s_all_t = [stage.tile([P, 64], f32, name=f"s_all{q}",
                                  tag=f"s_all{q}") for q in range(2)]
            ep = {}

            def ep_merge_ln(q):
                ca, cv = QTR_CT[q]["A"], QTR_CT[q]["V"]
                sa_lo, sv_lo = QTR_BASE[q]["A"], QTR_BASE[q]["V"]
                s_all = s_all_t[q]
                nc.scalar.copy(s_all[:, 0:ca], s_acc[:, sa_lo : sa_lo + ca])
                nc.scalar.copy(
                    s_all[:, ca : ca + cv], s_vec[:, sv_lo : sv_lo + cv]
                )
                # (for q == 1 the T-lane sums were already DMA'd into
                # s_all[:, 16:64])
                ln_s = stage.tile([P, 64], f32, tag=f"ln_s{q}")
                nc.scalar.activation(out=ln_s[:], in_=s_all[:], func=Act.Ln)
                ep[q] = dict(ln_s=ln_s)

            def ep_logpt(q):
                ln_s = ep[q]["ln_s"]
                logpt = stage.tile([P, 64], f32, tag=f"logpt{q}")
                dve_absorb(ln_s[:, 0:1])
                nc.vector.tensor_tensor(
                    out=logpt[:], in0=xt_all[:, q * 64 : (q + 1) * 64],
                    in1=ln_s[:], op=Alu.subtract,
                )
                ep[q]["logpt"] = logpt

            def ep_pt(q):
                logpt = ep[q]["logpt"]
                pt = stage.tile([P, 64], f32, tag=f"pt{q}")
                nc.scalar.activation(out=pt[:], in_=logpt[:], func=Act.Exp)
                om = stage.tile([P, 64], f32, tag=f"om{q}")  # 1 - pt
                nc.scalar.activation(
                    out=om[:], in_=pt[:], func=Act.Copy, scale=-1.0, bias=1.0
                )
                ln1m = stage.tile([P, 64], f32, tag=f"ln1m{q}")
                nc.scalar.activation(out=ln1m[:], in_=om[:], func=Act.Ln)
                gamma = stage.tile([P, 64], f32, tag=f"gamma{q}")
                mtmp = stage.tile([P, 64], f32, tag=f"mtmp{q}")
                ep[q].update(pt=pt, ln1m=ln1m, gamma=gamma, mtmp=mtmp)

            def ep_gamma(q, bins):
                st = ep[q]
                if bins[0] == 0:
                    nc.vector.memset(st["gamma"][:], gammas[0])
                for i in bins:
                    dg = gammas[i + 1] - gammas[i]
                    nc.vector.tensor_scalar(
                        out=st["mtmp"][:], in0=st["pt"][:],
                        scalar1=edges[i], scalar2=dg,
                        op0=Alu.is_ge, op1=Alu.mult,
                    )
                    nc.vector.tensor_tensor(
                        out=st["gamma"][:], in0=st["gamma"][:],
                        in1=st["mtmp"][:], op=Alu.add,
                    )

            def ep_suffix(q):
                st = ep[q]
                prod = stage.tile([P, 64], f32, tag=f"prod{q}")
                dve_absorb(st["ln1m"][:, 0:1])
                nc.vector.tensor_tensor(
                    out=prod[:], in0=st["gamma"][:], in1=st["ln1m"][:],
                    op=Alu.mult,
                )
                focal = stage.tile([P, 64], f32, tag=f"focal{q}")
                nc.scalar.activation(out=focal[:], in_=prod[:], func=Act.Exp)
                contrib = stage.tile([P, 64], f32, tag=f"contrib{q}")
                dve_absorb(focal[:, 0:1])
                nc.vector.tensor_tensor(
                    out=contrib[:], in0=focal[:], in1=st["logpt"][:],
                    op=Alu.mult,
                )
                nc.vector.tensor_reduce(
                    out=parts[:, q : q + 1], in_=contrib[:],
                    axis=mybir.AxisListType.X, op=Alu.add, negate=True,
                )

            def t_group(g):
                xT_t = xTpool.tile([P, CG, 512], bf16, tag="xT")
                src = xT_d[g * P * CG * 512 : (g + 1) * P * CG * 512]
                nc.sync.dma_start(
                    out=xT_t[:], in_=src.rearrange("(p x) -> p x", p=P)
                )
                # DVE sink absorbs the DMA wait so the tensor_scalar can
                # carry just its iT-recycle (PE) wait.
                sk = scratch.tile([P, 1], f32, tag="sink")
                nc.vector.tensor_tensor(
                    out=sk[:], in0=xT_t[:, 0, 0:2].bitcast(f32),
                    in1=ones[:], op=Alu.add,
                )
                iT_t = iTpool.tile([P, CG, 512], i16, tag="iT")
                nc.vector.tensor_scalar(
                    out=iT_t[:], in0=xT_t[:],
                    scalar1=SCH_C1, scalar2=SCH_C2,
                    op0=Alu.mult, op1=Alu.add,
                )
                for cg in range(CG):
                    nc.tensor.matmul(
                        psum_t[:],
                        wts[:, g, :],
                        iT_t[:, cg, :].bitcast(bf16),
                        start=(g == 0 and cg == 0),
                        stop=(g == TG - 1 and cg == CG - 1),
                    )

            for k in range(CHUNKS):
                ch = ASSIGN[k * J : (k + 1) * J]
                nd_k = ch.count("D")
                na_k = ch.count("A")
                if na_k:
                    x8_t = x8pool.tile([P, 3, C], fp8, tag="x8")
                    src = x8_d[a_start[k] * P * C : a_start[k + 1] * P * C]
                    src = src.rearrange("(p x) -> p x", p=P)
                    nc.sync.dma_start(out=x8_t[:, 0:na_k, :], in_=src)
                if nd_k:
                    x16_t = x16pool.tile([P, 2, C], bf16, tag="x16")
                    src = x16_d[b_start[k] * P * C : b_start[k + 1] * P * C]
                    src = src.rearrange("(p x) -> p x", p=P)
                    nc.sync.dma_start(out=x16_t[:, 0:nd_k, :], in_=src)
                    e_t = ipoolD.tile([P, 2, C], i16, tag="schD")
                    for j in range(nd_k):
                        nc.vector.tensor_scalar(
                            out=e_t[:, j, :], in0=x16_t[:, j, :],
                            scalar1=SCH_C1, scalar2=SCH_C2,
                            op0=Alu.mult, op1=Alu.add,
                        )
                    m0 = ENG_IDX[k * J]
                    nc.vector.tensor_reduce(
                        out=s_vec[:, m0 : m0 + nd_k],
                        in_=e_t[:, 0:nd_k, :].bitcast(bf16),
                        axis=mybir.AxisListType.X, op=Alu.add,
                    )
                for j in range(na_k):
                    m = ENG_IDX[k * J + nd_k + j]
                    dump = epool.tile([P, C], bf16, tag="exp_out")
                    nc.scalar.activation(
                        out=dump[:], in_=x8_t[:, j, :], func=Act.Exp,
                        accum_out=s_acc[:, m : m + 1],
                    )
                if k < TG:
                    t_group(k)
                if k == 16:
                    ep_merge_ln(0)
                elif k == 17:
                    ep_logpt(0)
                    ep_pt(0)
                elif k in (18, 19):
                    ep_gamma(0, range(7 * (k - 18), 7 * (k - 17)))

            ep_suffix(0)
            # T-lane: evacuate PSUM, re-tile via DRAM into half-1 staging
            flat_T = stage.tile([TG, 512], f32, tag="flatT")
            nc.vector.tensor_copy(out=flat_T[:], in_=psum_t[:])
            nc.sync.dma_start(
                out=scr_d[:].rearrange("(g f) -> g f", g=TG), in_=flat_T[:]
            )
            nc.sync.dma_start(
                out=s_all_t[1][:, H1_STREAM:64],
                in_=scr_d[:].rearrange("(p m) -> p m", p=P),
            )
            ep_merge_ln(1)
            ep_logpt(1)
            ep_pt(1)
            ep_gamma(1, range(NUM_BINS - 1))
            ep_suffix(1)
            part = stage.tile([P, 1], f32, tag="part")
            nc.vector.tensor_tensor(
                out=part[:], in0=parts[:, 0:1], in1=parts[:, 1:2], op=Alu.add
            )
            nc.sync.dma_start(out=out_d[:, :], in_=part[:])

    if hw_fixups:
        apply_hw_fixups(nc, mybir)
        verify_single_wait(nc)
    return nc


def apply_hw_fixups(nc, mybir):
    # Strip redundant own-engine waits (in-order queues make them no-ops).
    own_prefix = {
        "EngineType.DVE": "DVE",
        "EngineType.Activation": "Activation",
        "EngineType.Pool": "Pool",
        "EngineType.PE": "PE",
        "EngineType.SP": "SP",
    }
    for blk in nc.m.functions[0].blocks:
        for ins in blk.instructions:
            si = getattr(ins, "sync_info", None)
            if si is None or type(ins).__name__ == "InstDMACopy":
                continue
            if len(si.on_wait) <= 1:
                continue
            pref = own_prefix.get(str(getattr(ins, "engine", "")), None)
            if pref is None:
                continue
            keep = [w for w in si.on_wait if not w.ant_name.startswith(pref + "_")]
            if len(keep) < len(si.on_wait):
                ins.sync_info = type(si)(on_wait=keep, on_update=list(si.on_update))

    # Structural two-wait cases with a transitive single-wait replacement:
    #  - DVE tensor_scalar {PE WAR, DMA RAW} (T-lane): the sink absorbed
    #    the DMA wait for the DVE queue; keep the PE wait.
    for blk in nc.m.functions[0].blocks:
        for ins in blk.instructions:
            si = getattr(ins, "sync_info", None)
            if si is None or type(ins).__name__ == "InstDMACopy":
                continue
            if len(si.on_wait) <= 1:
                continue
            eng = str(getattr(ins, "engine", ""))
            names = [w.ant_name for w in si.on_wait]
            if (
                eng == "EngineType.DVE"
                and len(si.on_wait) == 2
                and any(n.startswith("PE") for n in names)
                and any(n.startswith("DMA") for n in names)
            ):
                keep = [w for w in si.on_wait if w.ant_name.startswith("PE")]
                ins.sync_info = type(si)(on_wait=keep, on_update=list(si.on_update))

    # walrus' DMA encoding holds a single sync wait. The last reader of a
    # stream slot is a DVE op for x16/xT and an ACT op for x8 (the T-lane
    # xT slots are last read by the PE matmuls, whose wait implies the DVE
    # tensor_scalar and its DMA wait).
    for blk in nc.m.functions[0].blocks:
        for ins in blk.instructions:
            si = getattr(ins, "sync_info", None)
            if si is None or type(ins).__name__ != "InstDMACopy":
                continue
            if len(si.on_wait) <= 1:
                continue
            own_lane = si.on_update[0].ant_name if si.on_update else ""
            keep = (
                [w for w in si.on_wait if w.ant_name.startswith("PE")]
                or [w for w in si.on_wait if w.ant_name.startswith("DVE")]
                or [w for w in si.on_wait if w.ant_name.startswith("Activation")]
                # DMA-producer RAW (e.g. the DRAM re-tile bounce): keep the
                # foreign lane, drop the own-lane ordering wait (increments
                # are atomic adds; issue order per queue is FIFO anyway).
                or [w for w in si.on_wait if w.ant_name != own_lane]
            )
            assert len(keep) == 1, (ins.name, [w.ant_name for w in si.on_wait])
            ins.sync_info = type(si)(on_wait=keep, on_update=list(si.on_update))

    # Split multi-wait kernel-tail drains into single-wait chains.
    for blk in nc.m.functions[0].blocks:
        il = blk.instructions
        i = 0
        while i < len(il):
            ins = il[i]
            si = getattr(ins, "sync_info", None)
            if (
                si is not None
                and type(ins).__name__ == "InstDrain"
                and len(si.on_wait) > 1
            ):
                SyncInfo = type(si)
                waits = list(si.on_wait)
                for k, w in enumerate(waits[:-1]):
                    d = mybir.InstDrain(
                        name=f"{ins.name}-w{k}", ins=[], outs=[],
                        bass_is_fusable=False,
                    )
                    d.engine = ins.engine
                    d.sync_info = SyncInfo(on_wait=[w], on_update=[])
                    il.insert(i, d)
                    i += 1
                ins.sync_info = SyncInfo(
                    on_wait=[waits[-1]], on_update=list(si.on_update)
                )
            i += 1


def verify_single_wait(nc):
    """Build-time check of the walrus single-wait constraint."""
    bad = []
    for blk in nc.m.functions[0].blocks:
        for ins in blk.instructions:
            si = getattr(ins, "sync_info", None)
            if si is not None and len(si.on_wait) > 1:
                bad.append(
                    (ins.name, type(ins).__name__,
                     str(getattr(ins, "engine", "")),
                     [w.ant_name for w in si.on_wait])
                )
    assert not bad, f"multi-wait instructions after fixups: {bad}"


def make_in_maps(input, target):
    import ml_dtypes

    x = np.asarray(input, dtype=np.float32)
    t = np.asarray(target).astype(np.int64)
    xt = x[np.arange(N), t]

    in_maps = []
    for core in range(NCORES):
        xs = x[core * RPC : (core + 1) * RPC]
        blocks = xs.reshape(COLS, P, C)
        # chunked streams over the first SCOLS row-blocks
        x8_parts, x16_parts = [], []
        for k in range(CHUNKS):
            ch = ASSIGN[k * J : (k + 1) * J]
            cols = list(range(k * J, (k + 1) * J))
            a_cols = [c for c, e in zip(cols, ch) if e == "A"]
            b_cols = [c for c, e in zip(cols, ch) if e != "A"]
            if a_cols:
                x8_parts.append(blocks[a_cols].transpose(1, 0, 2).reshape(-1))
            if b_cols:
                x16_parts.append(blocks[b_cols].transpose(1, 0, 2).reshape(-1))
        x8 = np.concatenate(x8_parts).astype(ml_dtypes.float8_e4m3)
        x16 = np.concatenate(x16_parts).astype(ml_dtypes.bfloat16)
        # T-lane: last TBLK row-blocks transposed, c padded to 1024
        xpad = np.full((TROWS, CPAD), -300.0, np.float32)
        xpad[:, :C] = xs[SCOLS * P :]
        xT = np.ascontiguousarray(
            xpad.T.reshape(CG, P, TG, 512).transpose(2, 1, 0, 3)
        ).astype(ml_dtypes.bfloat16)  # [g, p, cg, f]
        # xt layout matching the epilogue staging
        xts = np.zeros((P, COLS), np.float32)
        xt_core = xt[core * RPC : (core + 1) * RPC]
        xt_blk = xt_core[: SCOLS * P].reshape(SCOLS, P)
        for c in range(SCOLS):
            xts[:, epilogue_col(c)] = xt_blk[c]
        # T-row i sits at s_all1[i // TBLK, H1_STREAM + i % TBLK]
        xt_T = xt_core[SCOLS * P :].reshape(P, TBLK)
        xts[:, 64 + H1_STREAM : 128] = xt_T
        in_maps.append({"x8": x8, "x16": x16, "xT": xT.reshape(-1),
                        "xt": xts})
    return in_maps


def kernel(input, target, bin_edges, bin_gammas):
    global LAST_RESULT
    from concourse.bass_utils import run_bass_kernel_spmd

    nc = build_program(bin_edges, bin_gammas)
    in_maps = make_in_maps(input, target)
    trace = bool(os.environ.get("BASS_TRACE"))
    res = run_bass_kernel_spmd(nc, in_maps, list(range(NCORES)), trace=trace)
    LAST_RESULT = res
    total = np.float64(0.0)
    for r in res.results:
        total += r["out"].astype(np.float64).sum()
    return np.float32(total)
